# revision 2
# baseline (speedup 1.0000x reference)
"""Trainium2 Bass kernel for the DecoderLSTM problem.

Contract: kernel(**inputs) takes the FULL inputs (context, target_tensor,
coin, embed_table, W_ih, W_hh, b_ih, b_hh, W_out, b_out) and returns the
full (outs, h, c) like the reference.

Strategy (v1): data-parallel over batch across 8 cores (batch 16 per core,
replicated weights, sequential time loop per core, no collectives).

Device-side layout: activations keep batch on the SBUF partition dim;
weights are streamed as the matmul moving operand (rhs), activations^T are
the stationary operand.  Algebraic folds done on host:
  - gates = G0[b] + GE[tok_b] + h @ W_hh.T, where
      G0 = context @ W_ih[:, :C].T + b_ih + b_hh   (loop-invariant)
      GE = embed_table @ W_ih[:, C:].T             (embedding+input matmul)
  - G0/GE/b_out additions happen inside PSUM via identity/ones matmuls.
Gate rows are permuted host-side to [i, f, o, g] so sigmoid covers one
contiguous [0:1536] span and tanh covers [1536:2048].
"""

import numpy as np

import concourse.bass as bass
import concourse.tile as tile
from concourse import bacc, mybir
from concourse import bass_utils

B, T_FULL, V = 128, 256, 1000
C = H = E = 512
G4 = 4 * H  # 2048
N_CORES = 8
PB = B // N_CORES  # per-core batch

F32 = mybir.dt.float32
I32 = mybir.dt.int32


def _build_program(greedy, t_steps):
    """greedy: tuple of bools per step (True -> idx comes from argmax of the
    previous step's logits; False -> idx is host-provided in `tid`)."""
    nc = bacc.Bacc(
        "TRN2", target_bir_lowering=False, debug=False, num_devices=N_CORES
    )

    ge_d = nc.dram_tensor("ge", [V, G4], F32, kind="ExternalInput").ap()
    g0_d = nc.dram_tensor("g0", [PB, G4], F32, kind="ExternalInput").ap()
    whhT_d = nc.dram_tensor("whhT", [H, G4], F32, kind="ExternalInput").ap()
    woT_d = nc.dram_tensor("woT", [H, V], F32, kind="ExternalInput").ap()
    bout_d = nc.dram_tensor("bout", [1, V], F32, kind="ExternalInput").ap()
    ctxT_d = nc.dram_tensor("ctxT", [H, PB], F32, kind="ExternalInput").ap()
    tid_d = nc.dram_tensor("tid", [PB, t_steps], I32, kind="ExternalInput").ap()
    rev_d = nc.dram_tensor("rev", [PB, V], F32, kind="ExternalInput").ap()
    eye_d = nc.dram_tensor("eye", [PB, PB], F32, kind="ExternalInput").ap()
    ones_d = nc.dram_tensor("ones", [1, PB], F32, kind="ExternalInput").ap()

    out_lg_d = nc.dram_tensor(
        "out_lg", [PB, t_steps, V], F32, kind="ExternalOutput"
    ).ap()
    out_h_d = nc.dram_tensor("out_h", [PB, H], F32, kind="ExternalOutput").ap()
    out_c_d = nc.dram_tensor("out_c", [PB, H], F32, kind="ExternalOutput").ap()

    with tile.TileContext(nc) as tc:
        with (
            tc.tile_pool(name="const", bufs=1) as constp,
            tc.tile_pool(name="state", bufs=1) as statep,
            tc.tile_pool(name="hTp", bufs=2) as hTp,
            tc.tile_pool(name="gep", bufs=3) as gep,
            tc.tile_pool(name="work", bufs=2) as workp,
            tc.tile_pool(name="lgp", bufs=2) as lgp,
            tc.tile_pool(name="psg", bufs=1, space="PSUM") as psg,
            tc.tile_pool(name="psl", bufs=1, space="PSUM") as psl,
            tc.tile_pool(name="pst", bufs=2, space="PSUM") as pst,
        ):
            # ---- constants into SBUF ----
            whhT_sb = constp.tile([128, 4 * G4], F32)
            for k in range(4):
                nc.sync.dma_start(
                    whhT_sb[:, G4 * k : G4 * (k + 1)],
                    whhT_d[128 * k : 128 * (k + 1), :],
                )
            woT_sb = constp.tile([128, 4 * V], F32)
            for k in range(4):
                nc.sync.dma_start(
                    woT_sb[:, V * k : V * (k + 1)],
                    woT_d[128 * k : 128 * (k + 1), :],
                )
            g0_sb = constp.tile([PB, G4], F32)
            nc.sync.dma_start(g0_sb[:], g0_d[:])
            rev_sb = constp.tile([PB, V], F32)
            nc.sync.dma_start(rev_sb[:], rev_d[:])
            eye_sb = constp.tile([PB, PB], F32)
            nc.sync.dma_start(eye_sb[:], eye_d[:])
            ones_sb = constp.tile([1, PB], F32)
            nc.sync.dma_start(ones_sb[:], ones_d[:])
            bout_sb = constp.tile([1, V], F32)
            nc.sync.dma_start(bout_sb[:], bout_d[:])
            tid_sb = constp.tile([PB, t_steps], I32)
            nc.sync.dma_start(tid_sb[:], tid_d[:])

            c_sb = statep.tile([PB, H], F32)
            nc.vector.memset(c_sb[:], 0.0)

            # initial h^T = context^T (chunk k holds h dims [128k, 128k+128))
            hT = hTp.tile([128, 4 * PB], F32, tag="hT")
            for k in range(4):
                nc.sync.dma_start(
                    hT[:, PB * k : PB * (k + 1)],
                    ctxT_d[128 * k : 128 * (k + 1), :],
                )

            lg_prev = None
            h_sb = None
            for t in range(t_steps):
                # ---- token index for this step ----
                if greedy[t]:
                    mx = workp.tile([PB, 1], F32, tag="mx")
                    nc.vector.tensor_reduce(
                        mx[:], lg_prev[:], axis=mybir.AxisListType.X,
                        op=mybir.AluOpType.max,
                    )
                    m = workp.tile([PB, V], F32, tag="eqm")
                    nc.vector.scalar_tensor_tensor(
                        m[:], lg_prev[:], mx[:, 0:1], rev_sb[:],
                        op0=mybir.AluOpType.is_equal,
                        op1=mybir.AluOpType.mult,
                    )
                    r = workp.tile([PB, 1], F32, tag="r")
                    nc.vector.tensor_reduce(
                        r[:], m[:], axis=mybir.AxisListType.X,
                        op=mybir.AluOpType.max,
                    )
                    idxi = workp.tile([PB, 1], I32, tag="idxi")
                    nc.vector.tensor_scalar(
                        idxi[:], r[:], -1.0, float(V),
                        op0=mybir.AluOpType.mult, op1=mybir.AluOpType.add,
                    )
                    off_ap = idxi[:, 0:1]
                else:
                    off_ap = tid_sb[:, t : t + 1]

                ge_t = gep.tile([PB, G4], F32, tag="ge")
                nc.gpsimd.indirect_dma_start(
                    out=ge_t[:],
                    out_offset=None,
                    in_=ge_d[:],
                    in_offset=bass.IndirectOffsetOnAxis(ap=off_ap, axis=0),
                )

                # ---- gates = h @ whh^T  (+ G0 + GE via identity matmuls) ----
                gates_ps = psg.tile([PB, G4], F32, tag="g")
                for k in range(4):
                    lhsT = hT[:, PB * k : PB * (k + 1)]
                    for j in range(4):
                        nc.tensor.matmul(
                            gates_ps[:, 512 * j : 512 * (j + 1)],
                            lhsT,
                            whhT_sb[:, G4 * k + 512 * j : G4 * k + 512 * (j + 1)],
                            start=(k == 0),
                            stop=False,
                        )
                for j in range(4):
                    nc.tensor.matmul(
                        gates_ps[:, 512 * j : 512 * (j + 1)],
                        eye_sb[:],
                        g0_sb[:, 512 * j : 512 * (j + 1)],
                        start=False,
                        stop=False,
                    )
                for j in range(4):
                    nc.tensor.matmul(
                        gates_ps[:, 512 * j : 512 * (j + 1)],
                        eye_sb[:],
                        ge_t[:, 512 * j : 512 * (j + 1)],
                        start=False,
                        stop=True,
                    )

                # ---- nonlinearities ----
                act = workp.tile([PB, G4], F32, tag="act")
                nc.scalar.activation(
                    act[:, 0:1536], gates_ps[:, 0:1536],
                    mybir.ActivationFunctionType.Sigmoid,
                )
                nc.scalar.activation(
                    act[:, 1536:2048], gates_ps[:, 1536:2048],
                    mybir.ActivationFunctionType.Tanh,
                )

                # c = f*c + i*tanh(g);  h = o * tanh(c)
                t1 = workp.tile([PB, H], F32, tag="t1")
                nc.vector.tensor_tensor(
                    t1[:], act[:, 512:1024], c_sb[:], op=mybir.AluOpType.mult
                )
                t2 = workp.tile([PB, H], F32, tag="t2")
                nc.vector.tensor_tensor(
                    t2[:], act[:, 0:512], act[:, 1536:2048],
                    op=mybir.AluOpType.mult,
                )
                nc.vector.tensor_add(c_sb[:], t1[:], t2[:])
                tc_sb = workp.tile([PB, H], F32, tag="tc")
                nc.scalar.activation(
                    tc_sb[:], c_sb[:], mybir.ActivationFunctionType.Tanh
                )
                h_sb = workp.tile([PB, H], F32, tag="h")
                nc.vector.tensor_tensor(
                    h_sb[:], act[:, 1024:1536], tc_sb[:],
                    op=mybir.AluOpType.mult,
                )

                # ---- h^T via PE transpose ----
                tr_ps = pst.tile([128, 4 * PB], F32, tag="tr")
                for k in range(4):
                    nc.tensor.transpose(
                        tr_ps[:, PB * k : PB * (k + 1)],
                        h_sb[:, 128 * k : 128 * (k + 1)],
                        eye_sb[:],
                    )
                hT = hTp.tile([128, 4 * PB], F32, tag="hT")
                nc.vector.tensor_copy(hT[:], tr_ps[:])

                # ---- logits = h @ woT + b_out ----
                lg_ps = psl.tile([PB, V], F32, tag="lg")
                for k in range(4):
                    lhsT = hT[:, PB * k : PB * (k + 1)]
                    for j0, jn in ((0, 512), (512, 488)):
                        nc.tensor.matmul(
                            lg_ps[:, j0 : j0 + jn],
                            lhsT,
                            woT_sb[:, V * k + j0 : V * k + j0 + jn],
                            start=(k == 0),
                            stop=False,
                        )
                for j0, jn in ((0, 512), (512, 488)):
                    nc.tensor.matmul(
                        lg_ps[:, j0 : j0 + jn],
                        ones_sb[:],
                        bout_sb[:, j0 : j0 + jn],
                        start=False,
                        stop=True,
                    )
                lg_sb = lgp.tile([PB, V], F32, tag="lgs")
                nc.scalar.copy(lg_sb[:], lg_ps[:])
                nc.sync.dma_start(out_lg_d[:, t, :], lg_sb[:])
                lg_prev = lg_sb

            nc.sync.dma_start(out_h_d[:], h_sb[:])
            nc.sync.dma_start(out_c_d[:], c_sb[:])

    nc.compile()
    return nc


def _host_prep(context, target_tensor, coin, embed_table, W_ih, W_hh,
               b_ih, b_hh, W_out, b_out, t_steps):
    """Numpy-only input prep: layout permutes, loop-invariant folds, shards."""
    f32 = np.float32
    context = np.asarray(context, f32)
    target = np.asarray(target_tensor)
    coin = np.asarray(coin, f32)
    table = np.asarray(embed_table, f32)
    W_ih = np.asarray(W_ih, f32)
    W_hh = np.asarray(W_hh, f32)
    b_ih = np.asarray(b_ih, f32)
    b_hh = np.asarray(b_hh, f32)
    W_out = np.asarray(W_out, f32)
    b_out = np.asarray(b_out, f32)

    # permute gate rows: torch order i,f,g,o -> i,f,o,g
    perm = np.concatenate(
        [np.arange(0, 512), np.arange(512, 1024),
         np.arange(1536, 2048), np.arange(1024, 1536)]
    )
    Wih_p = W_ih[perm]
    Whh_p = W_hh[perm]
    bias_p = (b_ih.astype(np.float64) + b_hh.astype(np.float64))[perm]

    GE = (table.astype(np.float64) @ Wih_p[:, C:].T.astype(np.float64)).astype(f32)
    G0 = (
        context.astype(np.float64) @ Wih_p[:, :C].T.astype(np.float64) + bias_p
    ).astype(f32)
    whhT = np.ascontiguousarray(Whh_p.T)
    woT = np.ascontiguousarray(W_out.T)

    # teacher / start indices per step (int32); step 0 is always START=0
    tid = np.zeros((B, t_steps), np.int32)
    for t in range(1, t_steps):
        if coin[t] > 0.5:
            tid[:, t] = target[:, t - 1].astype(np.int32)
    greedy = tuple(bool(t > 0 and coin[t] <= 0.5) for t in range(t_steps))

    rev = np.tile((V - np.arange(V)).astype(f32), (PB, 1))
    eye = np.eye(PB, dtype=f32)
    ones = np.ones((1, PB), f32)
    bout = b_out.reshape(1, V)

    in_maps = []
    for cidx in range(N_CORES):
        sl = slice(cidx * PB, (cidx + 1) * PB)
        in_maps.append(
            {
                "ge": GE,
                "g0": np.ascontiguousarray(G0[sl]),
                "whhT": whhT,
                "woT": woT,
                "bout": bout,
                "ctxT": np.ascontiguousarray(context[sl].T),
                "tid": np.ascontiguousarray(tid[sl]),
                "rev": rev,
                "eye": eye,
                "ones": ones,
            }
        )
    return in_maps, greedy


_cache = {}


def _get_program(greedy, t_steps):
    key = (greedy, t_steps)
    if key not in _cache:
        _cache[key] = _build_program(greedy, t_steps)
    return _cache[key]


def run(t_steps=T_FULL, **inputs):
    in_maps, greedy = _host_prep(t_steps=t_steps, **inputs)
    nc = _get_program(greedy, t_steps)
    res = bass_utils.run_bass_kernel_spmd(
        nc, in_maps, core_ids=list(range(N_CORES))
    )
    outs = np.concatenate([res.results[c]["out_lg"] for c in range(N_CORES)], 0)
    h = np.concatenate([res.results[c]["out_h"] for c in range(N_CORES)], 0)
    c = np.concatenate([res.results[c]["out_c"] for c in range(N_CORES)], 0)
    return outs, h, c


def kernel(**inputs):
    return run(T_FULL, **inputs)


# revision 7
# speedup vs baseline: 6.4477x; 6.4477x over previous
"""Trainium2 Bass kernel for the DecoderLSTM problem.

Contract: kernel(**inputs) takes the FULL inputs (context, target_tensor,
coin, embed_table, W_ih, W_hh, b_ih, b_hh, W_out, b_out) and returns the
full (outs, h, c) like the reference.

Strategy (v1): data-parallel over batch across 8 cores (batch 16 per core,
replicated weights, sequential time loop per core, no collectives).

Device-side layout: activations keep batch on the SBUF partition dim;
weights are streamed as the matmul moving operand (rhs), activations^T are
the stationary operand.  Algebraic folds done on host:
  - gates = G0[b] + GE[tok_b] + h @ W_hh.T, where
      G0 = context @ W_ih[:, :C].T + b_ih + b_hh   (loop-invariant)
      GE = embed_table @ W_ih[:, C:].T             (embedding+input matmul)
  - G0/GE/b_out additions happen inside PSUM via identity/ones matmuls.
Gate rows are permuted host-side to [i, f, o, g] so sigmoid covers one
contiguous [0:1536] span and tanh covers [1536:2048].
"""

import numpy as np

import concourse.bass as bass
import concourse.tile as tile
from concourse import bacc, mybir
from concourse import bass_utils

B, T_FULL, V = 128, 256, 1000
C = H = E = 512
G4 = 4 * H  # 2048
N_CORES = 8
PB = B // N_CORES  # per-core batch

F32 = mybir.dt.float32
I32 = mybir.dt.int32


def _build_program(greedy, t_steps, ablate=frozenset()):
    """greedy: tuple of bools per step (True -> idx comes from argmax of the
    previous step's logits; False -> idx is host-provided in `tid`)."""
    nc = bacc.Bacc(
        "TRN2", target_bir_lowering=False, debug=False, num_devices=N_CORES
    )

    ge_d = nc.dram_tensor("ge", [V, G4], F32, kind="ExternalInput").ap()
    g0_d = nc.dram_tensor("g0", [PB, G4], F32, kind="ExternalInput").ap()
    whhT_d = nc.dram_tensor("whhT", [H, G4], F32, kind="ExternalInput").ap()
    woT_d = nc.dram_tensor("woT", [H, V], F32, kind="ExternalInput").ap()
    bout_d = nc.dram_tensor("bout", [PB, V], F32, kind="ExternalInput").ap()
    ctxT_d = nc.dram_tensor("ctxT", [H, PB], F32, kind="ExternalInput").ap()
    tid_d = nc.dram_tensor("tid", [PB, t_steps], I32, kind="ExternalInput").ap()
    rev_d = nc.dram_tensor("rev", [PB, V], F32, kind="ExternalInput").ap()
    eye_d = nc.dram_tensor("eye", [PB, PB], F32, kind="ExternalInput").ap()

    out_lg_d = nc.dram_tensor(
        "out_lg", [PB, t_steps, V], F32, kind="ExternalOutput"
    ).ap()
    out_h_d = nc.dram_tensor("out_h", [PB, H], F32, kind="ExternalOutput").ap()
    out_c_d = nc.dram_tensor("out_c", [PB, H], F32, kind="ExternalOutput").ap()

    with tile.TileContext(nc) as tc:
        with (
            tc.tile_pool(name="const", bufs=1) as constp,
            tc.tile_pool(name="state", bufs=1) as statep,
            tc.tile_pool(name="hTp", bufs=2) as hTp,
            tc.tile_pool(name="gep", bufs=3) as gep,
            tc.tile_pool(name="work", bufs=2) as workp,
            tc.tile_pool(name="lgp", bufs=2) as lgp,
            tc.tile_pool(name="psg", bufs=1, space="PSUM") as psg,
            tc.tile_pool(name="psl", bufs=1, space="PSUM") as psl,
            tc.tile_pool(name="pst", bufs=2, space="PSUM") as pst,
        ):
            # ---- constants into SBUF ----
            whhT_sb = constp.tile([128, 4 * G4], F32)
            for k in range(4):
                nc.sync.dma_start(
                    whhT_sb[:, G4 * k : G4 * (k + 1)],
                    whhT_d[128 * k : 128 * (k + 1), :],
                )
            woT_sb = constp.tile([128, 4 * V], F32)
            for k in range(4):
                nc.sync.dma_start(
                    woT_sb[:, V * k : V * (k + 1)],
                    woT_d[128 * k : 128 * (k + 1), :],
                )
            g0_sb = constp.tile([PB, G4], F32)
            nc.sync.dma_start(g0_sb[:], g0_d[:])
            rev_sb = constp.tile([PB, V], F32)
            nc.sync.dma_start(rev_sb[:], rev_d[:])
            eye_sb = constp.tile([PB, PB], F32)
            nc.sync.dma_start(eye_sb[:], eye_d[:])
            bout_sb = constp.tile([PB, V], F32)
            nc.sync.dma_start(bout_sb[:], bout_d[:])
            tid_sb = constp.tile([PB, t_steps], I32)
            nc.sync.dma_start(tid_sb[:], tid_d[:])

            c_sb = statep.tile([PB, H], F32)
            nc.vector.memset(c_sb[:], 0.0)

            # initial h^T = context^T (chunk k holds h dims [128k, 128k+128))
            hT = hTp.tile([128, 4 * PB], F32, tag="hT")
            for k in range(4):
                nc.sync.dma_start(
                    hT[:, PB * k : PB * (k + 1)],
                    ctxT_d[128 * k : 128 * (k + 1), :],
                )

            lg_prev = None
            h_sb = None
            for t in range(t_steps):
                # ---- token index for this step ----
                if greedy[t]:
                    mx = workp.tile([PB, 1], F32, tag="mx")
                    nc.vector.tensor_reduce(
                        mx[:], lg_prev[:], axis=mybir.AxisListType.X,
                        op=mybir.AluOpType.max,
                    )
                    m = workp.tile([PB, V], F32, tag="eqm")
                    nc.vector.scalar_tensor_tensor(
                        m[:], lg_prev[:], mx[:, 0:1], rev_sb[:],
                        op0=mybir.AluOpType.is_equal,
                        op1=mybir.AluOpType.mult,
                    )
                    r = workp.tile([PB, 1], F32, tag="r")
                    nc.vector.tensor_reduce(
                        r[:], m[:], axis=mybir.AxisListType.X,
                        op=mybir.AluOpType.max,
                    )
                    idxi = workp.tile([PB, 1], I32, tag="idxi")
                    nc.vector.tensor_scalar(
                        idxi[:], r[:], -1.0, float(V),
                        op0=mybir.AluOpType.mult, op1=mybir.AluOpType.add,
                    )
                    off_ap = idxi[:, 0:1]
                else:
                    off_ap = tid_sb[:, t : t + 1]

                ge_t = None
                if "gather" not in ablate:
                    ge_t = gep.tile([PB, G4], F32, tag="ge")
                    nc.gpsimd.indirect_dma_start(
                        out=ge_t[:],
                        out_offset=None,
                        in_=ge_d[:],
                        in_offset=bass.IndirectOffsetOnAxis(ap=off_ap, axis=0),
                    )

                # ---- gates = h @ whh^T  (+ G0 + GE via identity matmuls) ----
                gates_ps = psg.tile([PB, G4], F32, tag="g")
                nk = 1 if "whh" in ablate else 4
                for k in range(nk):
                    lhsT = hT[:, PB * k : PB * (k + 1)]
                    for j in range(4):
                        nc.tensor.matmul(
                            gates_ps[:, 512 * j : 512 * (j + 1)],
                            lhsT,
                            whhT_sb[:, G4 * k + 512 * j : G4 * k + 512 * (j + 1)],
                            start=(k == 0),
                            stop=(k == nk - 1),
                        )
                # mark last matmul group as stopped via a dummy: instead we
                # set stop on the final whh matmul by re-emitting? simpler:
                # the adds below read PSUM; stop flags handled on last k MM.
                pre = workp.tile([PB, G4], F32, tag="pre")
                nc.vector.tensor_add(pre[:], gates_ps[:], g0_sb[:])
                if ge_t is not None:
                    nc.vector.tensor_add(pre[:], pre[:], ge_t[:])

                # ---- nonlinearities ----
                act = workp.tile([PB, G4], F32, tag="act")
                nc.scalar.activation(
                    act[:, 0:1536], pre[:, 0:1536],
                    mybir.ActivationFunctionType.Sigmoid,
                )
                nc.scalar.activation(
                    act[:, 1536:2048], pre[:, 1536:2048],
                    mybir.ActivationFunctionType.Tanh,
                )

                # c = f*c + i*tanh(g);  h = o * tanh(c)
                t1 = workp.tile([PB, H], F32, tag="t1")
                nc.vector.tensor_tensor(
                    t1[:], act[:, 512:1024], c_sb[:], op=mybir.AluOpType.mult
                )
                t2 = workp.tile([PB, H], F32, tag="t2")
                nc.vector.tensor_tensor(
                    t2[:], act[:, 0:512], act[:, 1536:2048],
                    op=mybir.AluOpType.mult,
                )
                nc.vector.tensor_add(c_sb[:], t1[:], t2[:])
                tc_sb = workp.tile([PB, H], F32, tag="tc")
                nc.scalar.activation(
                    tc_sb[:], c_sb[:], mybir.ActivationFunctionType.Tanh
                )
                h_sb = workp.tile([PB, H], F32, tag="h")
                nc.vector.tensor_tensor(
                    h_sb[:], act[:, 1024:1536], tc_sb[:],
                    op=mybir.AluOpType.mult,
                )

                # ---- h^T via PE transpose ----
                if "transp" not in ablate:
                    tr_ps = pst.tile([128, 4 * PB], F32, tag="tr")
                    for k in range(4):
                        nc.tensor.transpose(
                            tr_ps[:, PB * k : PB * (k + 1)],
                            h_sb[:, 128 * k : 128 * (k + 1)],
                            eye_sb[:],
                        )
                    hT = hTp.tile([128, 4 * PB], F32, tag="hT")
                    nc.vector.tensor_copy(hT[:], tr_ps[:])

                # ---- logits = h @ woT + b_out ----
                lg_ps = psl.tile([PB, V], F32, tag="lg")
                nlk = 1 if "logits" in ablate else 4
                for k in range(nlk):
                    lhsT = hT[:, PB * k : PB * (k + 1)]
                    for j0, jn in ((0, 512), (512, 488)):
                        nc.tensor.matmul(
                            lg_ps[:, j0 : j0 + jn],
                            lhsT,
                            woT_sb[:, V * k + j0 : V * k + j0 + jn],
                            start=(k == 0),
                            stop=(k == nlk - 1),
                        )
                lg_sb = lgp.tile([PB, V], F32, tag="lgs")
                nc.vector.tensor_add(lg_sb[:], lg_ps[:], bout_sb[:])
                if "outdma" not in ablate:
                    nc.sync.dma_start(out_lg_d[:, t, :], lg_sb[:])
                lg_prev = lg_sb

            nc.sync.dma_start(out_h_d[:], h_sb[:])
            nc.sync.dma_start(out_c_d[:], c_sb[:])

    nc.compile()
    return nc


def _host_prep(context, target_tensor, coin, embed_table, W_ih, W_hh,
               b_ih, b_hh, W_out, b_out, t_steps):
    """Numpy-only input prep: layout permutes, loop-invariant folds, shards."""
    f32 = np.float32
    context = np.asarray(context, f32)
    target = np.asarray(target_tensor)
    coin = np.asarray(coin, f32)
    table = np.asarray(embed_table, f32)
    W_ih = np.asarray(W_ih, f32)
    W_hh = np.asarray(W_hh, f32)
    b_ih = np.asarray(b_ih, f32)
    b_hh = np.asarray(b_hh, f32)
    W_out = np.asarray(W_out, f32)
    b_out = np.asarray(b_out, f32)

    # permute gate rows: torch order i,f,g,o -> i,f,o,g
    perm = np.concatenate(
        [np.arange(0, 512), np.arange(512, 1024),
         np.arange(1536, 2048), np.arange(1024, 1536)]
    )
    Wih_p = W_ih[perm]
    Whh_p = W_hh[perm]
    bias_p = (b_ih.astype(np.float64) + b_hh.astype(np.float64))[perm]

    GE = (table.astype(np.float64) @ Wih_p[:, C:].T.astype(np.float64)).astype(f32)
    G0 = (
        context.astype(np.float64) @ Wih_p[:, :C].T.astype(np.float64) + bias_p
    ).astype(f32)
    whhT = np.ascontiguousarray(Whh_p.T)
    woT = np.ascontiguousarray(W_out.T)

    # teacher / start indices per step (int32); step 0 is always START=0
    tid = np.zeros((B, t_steps), np.int32)
    for t in range(1, t_steps):
        if coin[t] > 0.5:
            tid[:, t] = target[:, t - 1].astype(np.int32)
    greedy = tuple(bool(t > 0 and coin[t] <= 0.5) for t in range(t_steps))

    rev = np.tile((V - np.arange(V)).astype(f32), (PB, 1))
    eye = np.eye(PB, dtype=f32)
    bout = np.tile(b_out.reshape(1, V), (PB, 1))

    in_maps = []
    for cidx in range(N_CORES):
        sl = slice(cidx * PB, (cidx + 1) * PB)
        in_maps.append(
            {
                "ge": GE,
                "g0": np.ascontiguousarray(G0[sl]),
                "whhT": whhT,
                "woT": woT,
                "bout": bout,
                "ctxT": np.ascontiguousarray(context[sl].T),
                "tid": np.ascontiguousarray(tid[sl]),
                "rev": rev,
                "eye": eye,
            }
        )
    return in_maps, greedy


_cache = {}


def _get_program(greedy, t_steps):
    key = (greedy, t_steps)
    if key not in _cache:
        _cache[key] = _build_program(greedy, t_steps)
    return _cache[key]


def run(t_steps=T_FULL, **inputs):
    in_maps, greedy = _host_prep(t_steps=t_steps, **inputs)
    nc = _get_program(greedy, t_steps)
    res = bass_utils.run_bass_kernel_spmd(
        nc, in_maps, core_ids=list(range(N_CORES))
    )
    outs = np.concatenate([res.results[c]["out_lg"] for c in range(N_CORES)], 0)
    h = np.concatenate([res.results[c]["out_h"] for c in range(N_CORES)], 0)
    c = np.concatenate([res.results[c]["out_c"] for c in range(N_CORES)], 0)
    return outs, h, c


def kernel(**inputs):
    return run(T_FULL, **inputs)
